# revision 38
# baseline (speedup 1.0000x reference)
"""Asymmetric Hausdorff distance on 8 Trainium2 NeuronCores.

answer = max_i min_j ||pred[i,:3] - target[j,:3]||_2

Strategy (sharding_hint): shard pred rows across the 8 cores; each core
computes its row-block of the (implicit) distance matrix against the full
target set, takes per-row mins on the fly (never materializing the matrix),
then a global max over the 8 partial maxima (host-side, 8 scalars).

Per-core pipeline:
  d2(i,j) = |p_i|^2 + (|t_j|^2 - 2 p_i . t_j)
  s(i,j) = |t_j|^2 - 2 p_i . t_j is a K=11 bf16 matmul per (pred tile,
  target chunk), using a hi/lo bf16 split of each operand so the product
  is accurate to ~2^-16 (fp32 PSUM accumulate; the lo*lo term is dropped):
      lhsT rows: [a_hi(3), a_lo(3), a_hi(3), 1, 1]   a = -2*p
      rhs  rows: [t_hi(3), t_hi(3), t_lo(3), t2_hi, t2_lo]
  Matmuls run 4-way concurrent via tile_position row groups (targets are
  split into 4 subsets living at partition offsets 0/32/64/96), filling one
  4-bank PSUM tile per quad.  The rowwise min over PSUM is split across
  engines: some quads reduce directly on the Vector engine; the rest are
  drained by the Scalar engine to SBUF, pairwise-min'ed on GPSIMD, and
  min-reduced on Vector — balancing the three engines.  Then: add |p_i|^2,
  running max across pred tiles, cross-partition max via PE transpose,
  clamp+sqrt, one scalar out per core; host takes the max of 8.
"""

import numpy as np

import concourse.bass as bass
import concourse.mybir as mybir
import concourse.tile as tile
from concourse import bacc
from concourse.bass import ds
from concourse.bass_utils import run_bass_kernel_spmd
from concourse.masks import make_identity

F32 = mybir.dt.float32
BF16 = mybir.dt.bfloat16
AX = mybir.AxisListType
OP = mybir.AluOpType
ACT = mybir.ActivationFunctionType

N_CORES = 8
P = 128
KDIM = 11   # hi/lo split contraction: 3+3+3 products + t2_hi + t2_lo
MM_N = 512  # matmul moving chunk (one fp32 PSUM bank)
NSUB = 4    # concurrent row-group subsets (partition offsets 0/32/64/96)

# padded sizes: per-core pred rows (multiple of 128), target rows
# (multiple of NSUB*4*MM_N)
PRED_PAD = 3072   # 24 pred tiles of 128 per core
TGT_PAD = 24576   # 4 subsets x 12 chunks of 512; 12 quads per pred tile

# per-pred-tile quad schedule: which of the 12 quads reduce directly on DVE
# (fp32 from PSUM at 1 elem/cycle) vs go through the ACT-drain path (Scalar
# engine copies PSUM->SBUF bf16 with +|p|^2 bias, then DVE pairwise-mins at
# 2 elem/cycle in a tree and reduces once)
DIRECT_Q = (0, 6)
TREES_Q = ((1, 2, 3, 4), (5, 7, 8, 9), (10, 11))

LAST_RESULT = None  # BassKernelResults of the most recent run (for test.py)


def build_graph(pred_pad, tgt_pad, n_cores=N_CORES):
    assert pred_pad % P == 0
    assert tgt_pad % (NSUB * MM_N) == 0 and tgt_pad % P == 0
    n_ptiles = pred_pad // P
    n_tchunks = tgt_pad // P        # natural 128-row chunks
    n_quads = tgt_pad // (NSUB * MM_N)  # 4-bank PSUM tiles per pred tile
    if n_quads == 12:
        direct_q, trees_q = DIRECT_Q, TREES_Q
    elif n_quads >= 4:
        direct_q = (0,)
        trees_q = (tuple(range(1, n_quads)),)
    else:
        direct_q = tuple(range(n_quads))
        trees_q = ()

    nc = bacc.Bacc(trn_type="TRN2", num_devices=n_cores)

    pred_ext = nc.declare_dram_parameter("pred", [pred_pad, 4], F32, isOutput=False)
    tgt_ext = nc.declare_dram_parameter("target", [tgt_pad, 4], F32, isOutput=False)
    out_ext = nc.declare_dram_parameter("out", [1, 8], F32, isOutput=True)

    with tile.TileContext(nc) as tc:
        with (
            tc.tile_pool(name="big", bufs=1) as big,
            tc.tile_pool(name="work", bufs=3) as work,
            tc.tile_pool(name="drain", bufs=8) as drp,
            tc.tile_pool(name="pmain", bufs=2, space="PSUM") as pmain,
        ):
            identity = big.tile([P, P], BF16, tag="identity")
            make_identity(nc, identity[:])

            # ---- load inputs (row r of DRAM -> partition r // n_chunks,
            # chunk r % n_chunks: per-partition contiguous, fast DMA) ----
            pnat = big.tile([P, n_ptiles, 4], F32, tag="pnat")
            nc.sync.dma_start(
                out=pnat[:], in_=pred_ext[:].rearrange("(p c) k -> p c k", p=P)
            )
            tnat = big.tile([P, n_tchunks, 4], F32, tag="tnat")
            nc.sync.dma_start(
                out=tnat[:], in_=tgt_ext[:].rearrange("(p c) k -> p c k", p=P)
            )

            # ---- pred preprocessing ----
            # a = -2p; split a = a_hi + a_lo (bf16 each); p2 = |p|^2 (fp32)
            pa = big.tile([P, n_ptiles, 3], F32, tag="pa")
            nc.vector.tensor_scalar_mul(pa[:], pnat[:, :, 0:3], -2.0)
            pblk = big.tile([P, n_ptiles, KDIM], BF16, tag="pblk")
            a_hi = pblk[:, :, 0:3]
            nc.scalar.copy(a_hi, pa[:])                      # fp32 -> bf16 round
            pa_hi32 = big.tile([P, n_ptiles, 3], F32, tag="pa_hi32")
            nc.scalar.copy(pa_hi32[:], a_hi)                 # bf16 -> fp32 exact
            nc.vector.tensor_sub(pblk[:, :, 3:6], pa[:], pa_hi32[:])  # a_lo
            nc.vector.tensor_copy(pblk[:, :, 6:9], a_hi)     # a_hi again
            nc.vector.memset(pblk[:, :, 9:11], 1.0)
            psq = big.tile([P, n_ptiles, 3], F32, tag="psq")
            nc.vector.tensor_mul(psq[:], pnat[:, :, 0:3], pnat[:, :, 0:3])
            p2all = big.tile([P, n_ptiles], F32, tag="p2all")
            nc.vector.tensor_reduce(p2all[:], psq[:], axis=AX.X, op=OP.add)

            # ---- target preprocessing ----
            # t = t_hi + t_lo; t2 = |t|^2 = t2_hi + t2_lo (bf16 pairs)
            tblk = big.tile([P, n_tchunks, KDIM], BF16, tag="tblk")
            t_hi = tblk[:, :, 0:3]
            nc.scalar.copy(t_hi, tnat[:, :, 0:3])
            t_hi32 = big.tile([P, n_tchunks, 3], F32, tag="t_hi32")
            nc.scalar.copy(t_hi32[:], t_hi)
            nc.vector.tensor_sub(tblk[:, :, 6:9], tnat[:, :, 0:3], t_hi32[:])  # t_lo
            nc.vector.tensor_copy(tblk[:, :, 3:6], t_hi)
            tsq = big.tile([P, n_tchunks, 3], F32, tag="tsq")
            nc.vector.tensor_mul(tsq[:], tnat[:, :, 0:3], tnat[:, :, 0:3])
            t2 = big.tile([P, n_tchunks], F32, tag="t2")
            nc.vector.tensor_reduce(t2[:], tsq[:], axis=AX.X, op=OP.add)
            t2_hi = tblk[:, :, 9:10]
            nc.scalar.copy(t2_hi, t2[:].rearrange("p (c o) -> p c o", o=1))
            t2_hi32 = big.tile([P, n_tchunks], F32, tag="t2_hi32")
            nc.scalar.copy(t2_hi32[:].rearrange("p (c o) -> p c o", o=1), t2_hi)
            nc.vector.tensor_sub(
                tblk[:, :, 10:11],
                t2[:].rearrange("p (c o) -> p c o", o=1),
                t2_hi32[:].rearrange("p (c o) -> p c o", o=1),
            )

            # ---- transpose to matmul layout via PE ----
            # lhsT_sb [128, pred_pad]: pred tile c at cols 128c..128c+127,
            #   K rows replicated at partition offsets 0/32/64/96.
            # rhs_sb [128, tgt_pad/4]: natural chunk c lives in subset
            #   g = c%4 (partition offset 32g), chunk-col c//4.
            # Staging tiles use the full 4-bank main-pool slot ([128, 4096]
            # bf16): pred fits in one batch (24 col-slots), targets in
            # ceil(192/128) = 2 batches of up to 32 col-slots x 4 row-groups.
            SLOTW = 4096 // P  # 32 col-slots per staging tile
            lhsT_sb = big.tile([P, pred_pad], BF16, tag="lhsT")
            for b in range(0, n_ptiles, SLOTW):
                nb = min(SLOTW, n_ptiles - b)
                tr = pmain.tile([P, 4096], BF16, tag="ps")
                nc.vector.memset(tr[:].bitcast(F32), 0.0)
                for j in range(nb):
                    c = b + j
                    for g in range(NSUB):
                        nc.tensor.transpose(
                            tr[32 * g : 32 * g + KDIM, j * P : (j + 1) * P],
                            pblk[:, c, :],
                            identity[:],
                            tile_position=(0, 32 * g),
                        )
                nc.scalar.copy(
                    lhsT_sb[:, b * P : (b + nb) * P], tr[:, 0 : nb * P]
                )

            rhs_sb = big.tile([P, tgt_pad // NSUB], BF16, tag="rhs")
            for b in range(0, n_tchunks, 4 * SLOTW):
                nb = min(4 * SLOTW, n_tchunks - b)
                tr = pmain.tile([P, 4096], BF16, tag="ps")
                nc.vector.memset(tr[:].bitcast(F32), 0.0)
                for j in range(nb):
                    c = b + j
                    g = c % NSUB
                    slot = (c % (4 * SLOTW)) // NSUB
                    nc.tensor.transpose(
                        tr[32 * g : 32 * g + KDIM, slot * P : (slot + 1) * P],
                        tblk[:, c, :],
                        identity[:],
                        tile_position=(0, 32 * g),
                    )
                nc.scalar.copy(
                    rhs_sb[:, (b // 4) * P : (b // 4) * P + (nb // 4) * P],
                    tr[:, 0 : (nb // 4) * P],
                )

            # ---- main loop: 4-way concurrent matmuls + split min-reduce ----
            maxbuf = big.tile([P, n_ptiles], F32, tag="maxbuf")

            def quad_matmuls(c, q):
                ps = pmain.tile([P, NSUB * MM_N], F32, tag="ps")
                for g in range(NSUB):
                    nc.tensor.matmul(
                        ps[:, g * MM_N : (g + 1) * MM_N],
                        lhsT_sb[32 * g : 32 * g + KDIM, c * P : (c + 1) * P],
                        rhs_sb[32 * g : 32 * g + KDIM, q * MM_N : (q + 1) * MM_N],
                        start=True,
                        stop=True,
                        tile_position=(32 * g, 0),
                    )
                return ps

            bigA = big.tile([P, n_ptiles, max(len(direct_q), 1)], F32, tag="bigA")
            bigC = big.tile([P, n_ptiles, max(len(trees_q), 1)], F32, tag="bigC")
            for c in range(n_ptiles):
                p2c = p2all[:, ds(c, 1)]
                # route A: direct fp32 min-reduce from PSUM (values are s)
                # route C: ACT drains PSUM -> SBUF bf16 with +|p|^2 bias
                # (values are d2 >= 0, so bf16 keeps ~1e-3 relative), DVE
                # pairwise-mins at 2x in a tree, one bf16 reduce per tree.
                # Quads are emitted in index order (routes interleaved) so
                # the scheduler can overlap ACT drains with DVE reduces.
                minsA = bigA[:, c, :]
                minsC = bigC[:, c, :]
                tree_of_q = {}
                for ti, tree in enumerate(trees_q):
                    for q in tree:
                        tree_of_q[q] = ti
                levels = [[] for _ in trees_q]
                colA = 0
                for q in range(n_quads):
                    ps = quad_matmuls(c, q)
                    if q in direct_q:
                        nc.vector.tensor_reduce(
                            minsA[:, ds(colA, 1)], ps[:], axis=AX.X, op=OP.min
                        )
                        colA += 1
                        continue
                    ti = tree_of_q[q]
                    dr = drp.tile([P, NSUB * MM_N], BF16, tag="dr")
                    nc.scalar.activation(dr[:], ps[:], ACT.Identity, bias=p2c)
                    levels[ti].append(dr)
                    # combine pairs as soon as they are available
                    while len(levels[ti]) >= 2:
                        a, b = levels[ti][-2], levels[ti][-1]
                        mg = drp.tile([P, NSUB * MM_N], BF16, tag="mg")
                        nc.vector.tensor_tensor(mg[:], a[:], b[:], op=OP.min)
                        levels[ti] = levels[ti][:-2] + [mg]
                        if len(levels[ti]) < 2:
                            break
                for ti, tree in enumerate(trees_q):
                    level = levels[ti]
                    while len(level) > 1:
                        mg = drp.tile([P, NSUB * MM_N], BF16, tag="mg")
                        nc.vector.tensor_tensor(
                            mg[:], level[-2][:], level[-1][:], op=OP.min
                        )
                        level = level[:-2] + [mg]
                    nc.vector.tensor_reduce(
                        minsC[:, ds(ti, 1)], level[0][:], axis=AX.X, op=OP.min
                    )
                pass

            # batched combine: d2[p, c] = min(min_A + p2, min_C)
            dA = big.tile([P, n_ptiles], F32, tag="dA")
            nc.vector.tensor_reduce(dA[:], bigA[:], axis=AX.X, op=OP.min)
            nc.vector.tensor_add(dA[:], dA[:], p2all[:])
            if trees_q:
                dC = big.tile([P, n_ptiles], F32, tag="dC")
                nc.vector.tensor_reduce(dC[:], bigC[:], axis=AX.X, op=OP.min)
                nc.vector.tensor_tensor(maxbuf[:], dA[:], dC[:], op=OP.min)
            else:
                nc.vector.tensor_copy(maxbuf[:], dA[:])

            # ---- finalize: max over partitions (fp32 PE transpose), clamp,
            # sqrt; one scalar per core, host maxes the 8 ----
            identity32 = big.tile([P, P], F32, tag="identity32")
            make_identity(nc, identity32[:])
            gmax = big.tile([P, 1], F32, tag="gmax")
            nc.vector.tensor_reduce(gmax[:], maxbuf[:], axis=AX.X, op=OP.max)
            trf = pmain.tile([P, 512], F32, tag="ps")
            nc.tensor.transpose(trf[0:1, 0:P], gmax[:], identity32[:])
            grow = big.tile([1, P], F32, tag="grow")
            nc.scalar.copy(grow[:], trf[0:1, 0:P])
            gsc = big.tile([1, 1], F32, tag="gsc")
            nc.vector.tensor_reduce(gsc[:], grow[:], axis=AX.X, op=OP.max)
            gre = big.tile([1, 1], F32, tag="gre")
            nc.scalar.activation(gre[:], gsc[:], ACT.Relu)
            gsq = big.tile([1, 1], F32, tag="gsq")
            nc.scalar.sqrt(gsq[:], gre[:])
            fin = big.tile([1, 8], F32, tag="fin")
            nc.vector.memset(fin[:], 0.0)
            nc.scalar.copy(fin[:, 0:1], gsq[:])
            nc.sync.dma_start(out=out_ext[:], in_=fin[:])

    nc.finalize()
    return nc


def shard_inputs(pred, target, pred_pad=PRED_PAD, tgt_pad=TGT_PAD, n_cores=N_CORES):
    pred = np.ascontiguousarray(pred, dtype=np.float32)
    target = np.ascontiguousarray(target, dtype=np.float32)
    n_pred = pred.shape[0]
    n_tgt = target.shape[0]
    per = (n_pred + n_cores - 1) // n_cores
    tpad = np.empty((tgt_pad, 4), np.float32)
    tpad[:n_tgt] = target
    tpad[n_tgt:] = target[0]  # duplicate targets never change a min
    in_maps = []
    for i in range(n_cores):
        lo = min(i * per, n_pred)
        hi = min(lo + per, n_pred)
        shard = np.empty((pred_pad, 4), np.float32)
        shard[: hi - lo] = pred[lo:hi]
        shard[hi - lo :] = pred[lo if hi > lo else 0]  # duplicate real rows
        in_maps.append({"pred": shard, "target": tpad})
    return in_maps


_NC_CACHE = {}


def kernel(pred, target, trace=False):
    global LAST_RESULT
    key = (PRED_PAD, TGT_PAD)
    if key not in _NC_CACHE:
        _NC_CACHE[key] = build_graph(*key)
    nc = _NC_CACHE[key]
    in_maps = shard_inputs(pred, target)
    res = run_bass_kernel_spmd(nc, in_maps, core_ids=list(range(N_CORES)), trace=trace)
    LAST_RESULT = res
    # host-side "all-reduce": max over the 8 per-core partial maxima
    val = max(float(res.results[i]["out"][0, 0]) for i in range(N_CORES))
    return np.array(val, dtype=np.float32)


# revision 41
# speedup vs baseline: 1.0182x; 1.0182x over previous
"""Asymmetric Hausdorff distance on 8 Trainium2 NeuronCores.

answer = max_i min_j ||pred[i,:3] - target[j,:3]||_2

Strategy (sharding_hint): shard pred rows across the 8 cores; each core
computes its row-block of the (implicit) distance matrix against the full
target set, takes per-row mins on the fly (never materializing the matrix),
then a global max over the 8 partial maxima (host-side, 8 scalars).

Per-core pipeline:
  d2(i,j) = |p_i|^2 + (|t_j|^2 - 2 p_i . t_j)
  s(i,j) = |t_j|^2 - 2 p_i . t_j is a K=11 bf16 matmul per (pred tile,
  target chunk), using a hi/lo bf16 split of each operand so the product
  is accurate to ~2^-16 (fp32 PSUM accumulate; the lo*lo term is dropped):
      lhsT rows: [a_hi(3), a_lo(3), a_hi(3), 1, 1]   a = -2*p
      rhs  rows: [t_hi(3), t_hi(3), t_lo(3), t2_hi, t2_lo]
  Matmuls run 4-way concurrent via tile_position row groups (targets are
  split into 4 subsets living at partition offsets 0/32/64/96), filling one
  4-bank PSUM tile per quad.  The rowwise min over PSUM is split across
  engines: some quads reduce directly on the Vector engine; the rest are
  drained by the Scalar engine to SBUF, pairwise-min'ed on GPSIMD, and
  min-reduced on Vector — balancing the three engines.  Then: add |p_i|^2,
  running max across pred tiles, cross-partition max via PE transpose,
  clamp+sqrt, one scalar out per core; host takes the max of 8.
"""

import numpy as np

import concourse.bass as bass
import concourse.mybir as mybir
import concourse.tile as tile
from concourse import bacc
from concourse.bass import ds
from concourse.bass_utils import run_bass_kernel_spmd
from concourse.masks import make_identity

F32 = mybir.dt.float32
BF16 = mybir.dt.bfloat16
AX = mybir.AxisListType
OP = mybir.AluOpType
ACT = mybir.ActivationFunctionType

N_CORES = 8
P = 128
KDIM = 11   # hi/lo split contraction: 3+3+3 products + t2_hi + t2_lo
MM_N = 512  # matmul moving chunk (one fp32 PSUM bank)
NSUB = 4    # concurrent row-group subsets (partition offsets 0/32/64/96)

# padded sizes: per-core pred rows (multiple of 128), target rows
# (multiple of NSUB*4*MM_N)
PRED_PAD = 3072   # 24 pred tiles of 128 per core
TGT_PAD = 24576   # 4 subsets x 12 chunks of 512; 12 quads per pred tile

# per-pred-tile quad schedule: which of the 12 quads reduce directly on DVE
# (fp32 from PSUM at 1 elem/cycle) vs go through the ACT-drain path (Scalar
# engine copies PSUM->SBUF bf16 with +|p|^2 bias, then DVE pairwise-mins at
# 2 elem/cycle in a tree and reduces once)
DIRECT_Q = (0, 6)
TREES_Q = ((1, 2, 3, 4), (5, 7, 8, 9), (10, 11))

LAST_RESULT = None  # BassKernelResults of the most recent run (for test.py)


def build_graph(pred_pad, tgt_pad, n_cores=N_CORES):
    assert pred_pad % P == 0
    assert tgt_pad % (NSUB * MM_N) == 0 and tgt_pad % P == 0
    n_ptiles = pred_pad // P
    n_tchunks = tgt_pad // P        # natural 128-row chunks
    n_quads = tgt_pad // (NSUB * MM_N)  # 4-bank PSUM tiles per pred tile
    if n_quads == 12:
        direct_q, trees_q = DIRECT_Q, TREES_Q
    elif n_quads >= 4:
        direct_q = (0,)
        trees_q = (tuple(range(1, n_quads)),)
    else:
        direct_q = tuple(range(n_quads))
        trees_q = ()

    nc = bacc.Bacc(trn_type="TRN2", num_devices=n_cores)

    pred_ext = nc.declare_dram_parameter("pred", [pred_pad, 4], F32, isOutput=False)
    tgt_ext = nc.declare_dram_parameter("target", [tgt_pad, 4], F32, isOutput=False)
    out_ext = nc.declare_dram_parameter("out", [1, 8], F32, isOutput=True)

    with tile.TileContext(nc) as tc:
        with (
            tc.tile_pool(name="big", bufs=1) as big,
            tc.tile_pool(name="work", bufs=3) as work,
            tc.tile_pool(name="drain", bufs=8) as drp,
            tc.tile_pool(name="pmain", bufs=2, space="PSUM") as pmain,
        ):
            identity = big.tile([P, P], BF16, tag="identity")
            make_identity(nc, identity[:])

            # ---- load inputs (row r of DRAM -> partition r // n_chunks,
            # chunk r % n_chunks: per-partition contiguous, fast DMA) ----
            pnat = big.tile([P, n_ptiles, 4], F32, tag="pnat")
            nc.sync.dma_start(
                out=pnat[:], in_=pred_ext[:].rearrange("(p c) k -> p c k", p=P)
            )
            tnat = big.tile([P, n_tchunks, 4], F32, tag="tnat")
            nc.sync.dma_start(
                out=tnat[:], in_=tgt_ext[:].rearrange("(p c) k -> p c k", p=P)
            )

            # ---- pred preprocessing ----
            # a = -2p; split a = a_hi + a_lo (bf16 each); p2 = |p|^2 (fp32)
            pa = big.tile([P, n_ptiles, 3], F32, tag="pa")
            nc.vector.tensor_scalar_mul(pa[:], pnat[:, :, 0:3], -2.0)
            pblk = big.tile([P, n_ptiles, NSUB, 32], BF16, tag="pblk")
            nc.gpsimd.memset(pblk[:].bitcast(F32), 0.0)
            pblk0 = pblk[:, :, 0, :]
            a_hi = pblk0[:, :, 0:3]
            nc.scalar.copy(a_hi, pa[:])                      # fp32 -> bf16 round
            pa_hi32 = big.tile([P, n_ptiles, 3], F32, tag="pa_hi32")
            nc.scalar.copy(pa_hi32[:], a_hi)                 # bf16 -> fp32 exact
            nc.vector.tensor_sub(pblk0[:, :, 3:6], pa[:], pa_hi32[:])  # a_lo
            nc.vector.tensor_copy(pblk0[:, :, 6:9], a_hi)     # a_hi again
            nc.vector.memset(pblk0[:, :, 9:11], 1.0)
            for g in range(1, NSUB):  # replicate K block to all row groups
                nc.gpsimd.tensor_copy(pblk[:, :, g, :], pblk0[:])
            psq = big.tile([P, n_ptiles, 3], F32, tag="psq")
            nc.vector.tensor_mul(psq[:], pnat[:, :, 0:3], pnat[:, :, 0:3])
            p2all = big.tile([P, n_ptiles], F32, tag="p2all")
            nc.vector.tensor_reduce(p2all[:], psq[:], axis=AX.X, op=OP.add)

            # ---- target preprocessing ----
            # t = t_hi + t_lo; t2 = |t|^2 = t2_hi + t2_lo (bf16 pairs)
            tblk = big.tile([P, n_tchunks, 32], BF16, tag="tblk")
            nc.gpsimd.memset(tblk[:].bitcast(F32), 0.0)
            t_hi = tblk[:, :, 0:3]
            nc.scalar.copy(t_hi, tnat[:, :, 0:3])
            t_hi32 = big.tile([P, n_tchunks, 3], F32, tag="t_hi32")
            nc.scalar.copy(t_hi32[:], t_hi)
            nc.vector.tensor_sub(tblk[:, :, 6:9], tnat[:, :, 0:3], t_hi32[:])  # t_lo
            nc.vector.tensor_copy(tblk[:, :, 3:6], t_hi)
            tsq = big.tile([P, n_tchunks, 3], F32, tag="tsq")
            nc.vector.tensor_mul(tsq[:], tnat[:, :, 0:3], tnat[:, :, 0:3])
            t2 = big.tile([P, n_tchunks], F32, tag="t2")
            nc.vector.tensor_reduce(t2[:], tsq[:], axis=AX.X, op=OP.add)
            t2_hi = tblk[:, :, 9:10]
            nc.scalar.copy(t2_hi, t2[:].rearrange("p (c o) -> p c o", o=1))
            t2_hi32 = big.tile([P, n_tchunks], F32, tag="t2_hi32")
            nc.scalar.copy(t2_hi32[:].rearrange("p (c o) -> p c o", o=1), t2_hi)
            nc.vector.tensor_sub(
                tblk[:, :, 10:11],
                t2[:].rearrange("p (c o) -> p c o", o=1),
                t2_hi32[:].rearrange("p (c o) -> p c o", o=1),
            )

            # ---- transpose to matmul layout via PE ----
            # lhsT_sb [128, pred_pad]: pred tile c at cols 128c..128c+127,
            #   K rows replicated at partition offsets 0/32/64/96.
            # rhs_sb [128, tgt_pad/4]: natural chunk c lives in subset
            #   g = c%4 (partition offset 32g), chunk-col c//4.
            # Each [128, 128] transpose covers FOUR chunks: the 11 K-values
            # sit at columns 0-10 of a 32-col block, so transposing a
            # [128, 4x32] input lands chunk g at partition offset 32g --
            # exactly the row-group layout the concurrent matmuls need.
            # Pred replicates one chunk across all 4 groups via a stride-0
            # broadcast AP.  Staging tiles hold 32 transposes (4-bank slot).
            lhsT_sb = big.tile([P, pred_pad], BF16, tag="lhsT")
            for b in range(0, n_ptiles, 32):
                nb = min(32, n_ptiles - b)
                tr = pmain.tile([P, 4096], BF16, tag="ps")
                for j in range(nb):
                    c = b + j
                    nc.tensor.transpose(
                        tr[:, j * P : (j + 1) * P],
                        pblk[:, c, :, :],
                        identity[:],
                    )
                nc.scalar.copy(
                    lhsT_sb[:, b * P : (b + nb) * P], tr[:, 0 : nb * P]
                )

            rhs_sb = big.tile([P, tgt_pad // NSUB], BF16, tag="rhs")
            n_cc = n_tchunks // NSUB
            for b in range(0, n_cc, 32):
                nb = min(32, n_cc - b)
                tr = pmain.tile([P, 4096], BF16, tag="ps")
                for j in range(nb):
                    cc = b + j
                    nc.tensor.transpose(
                        tr[:, j * P : (j + 1) * P],
                        tblk[:, NSUB * cc : NSUB * (cc + 1), :],
                        identity[:],
                    )
                nc.scalar.copy(
                    rhs_sb[:, b * P : (b + nb) * P], tr[:, 0 : nb * P]
                )

            # ---- main loop: 4-way concurrent matmuls + split min-reduce ----
            maxbuf = big.tile([P, n_ptiles], F32, tag="maxbuf")

            def quad_matmuls(c, q):
                ps = pmain.tile([P, NSUB * MM_N], F32, tag="ps")
                for g in range(NSUB):
                    nc.tensor.matmul(
                        ps[:, g * MM_N : (g + 1) * MM_N],
                        lhsT_sb[32 * g : 32 * g + KDIM, c * P : (c + 1) * P],
                        rhs_sb[32 * g : 32 * g + KDIM, q * MM_N : (q + 1) * MM_N],
                        start=True,
                        stop=True,
                        tile_position=(32 * g, 0),
                    )
                return ps

            bigA = big.tile([P, n_ptiles, max(len(direct_q), 1)], F32, tag="bigA")
            bigC = big.tile([P, n_ptiles, max(len(trees_q), 1)], F32, tag="bigC")
            for c in range(n_ptiles):
                p2c = p2all[:, ds(c, 1)]
                # route A: direct fp32 min-reduce from PSUM (values are s)
                # route C: ACT drains PSUM -> SBUF bf16 with +|p|^2 bias
                # (values are d2 >= 0, so bf16 keeps ~1e-3 relative), DVE
                # pairwise-mins at 2x in a tree, one bf16 reduce per tree.
                # Quads are emitted in index order (routes interleaved) so
                # the scheduler can overlap ACT drains with DVE reduces.
                minsA = bigA[:, c, :]
                minsC = bigC[:, c, :]
                tree_of_q = {}
                for ti, tree in enumerate(trees_q):
                    for q in tree:
                        tree_of_q[q] = ti
                levels = [[] for _ in trees_q]
                colA = 0
                for q in range(n_quads):
                    ps = quad_matmuls(c, q)
                    if q in direct_q:
                        nc.vector.tensor_reduce(
                            minsA[:, ds(colA, 1)], ps[:], axis=AX.X, op=OP.min
                        )
                        colA += 1
                        continue
                    ti = tree_of_q[q]
                    dr = drp.tile([P, NSUB * MM_N], BF16, tag="dr")
                    nc.scalar.activation(dr[:], ps[:], ACT.Identity, bias=p2c)
                    levels[ti].append(dr)
                    # combine pairs as soon as they are available
                    while len(levels[ti]) >= 2:
                        a, b = levels[ti][-2], levels[ti][-1]
                        mg = drp.tile([P, NSUB * MM_N], BF16, tag="mg")
                        nc.vector.tensor_tensor(mg[:], a[:], b[:], op=OP.min)
                        levels[ti] = levels[ti][:-2] + [mg]
                        if len(levels[ti]) < 2:
                            break
                for ti, tree in enumerate(trees_q):
                    level = levels[ti]
                    while len(level) > 1:
                        mg = drp.tile([P, NSUB * MM_N], BF16, tag="mg")
                        nc.vector.tensor_tensor(
                            mg[:], level[-2][:], level[-1][:], op=OP.min
                        )
                        level = level[:-2] + [mg]
                    nc.vector.tensor_reduce(
                        minsC[:, ds(ti, 1)], level[0][:], axis=AX.X, op=OP.min
                    )
                pass

            # batched combine: d2[p, c] = min(min_A + p2, min_C)
            dA = big.tile([P, n_ptiles], F32, tag="dA")
            nc.vector.tensor_reduce(dA[:], bigA[:], axis=AX.X, op=OP.min)
            nc.vector.tensor_add(dA[:], dA[:], p2all[:])
            if trees_q:
                dC = big.tile([P, n_ptiles], F32, tag="dC")
                nc.vector.tensor_reduce(dC[:], bigC[:], axis=AX.X, op=OP.min)
                nc.vector.tensor_tensor(maxbuf[:], dA[:], dC[:], op=OP.min)
            else:
                nc.vector.tensor_copy(maxbuf[:], dA[:])

            # ---- finalize: max over partitions (fp32 PE transpose), clamp,
            # sqrt; one scalar per core, host maxes the 8 ----
            identity32 = big.tile([P, P], F32, tag="identity32")
            make_identity(nc, identity32[:])
            gmax = big.tile([P, 1], F32, tag="gmax")
            nc.vector.tensor_reduce(gmax[:], maxbuf[:], axis=AX.X, op=OP.max)
            trf = pmain.tile([P, 512], F32, tag="ps")
            nc.tensor.transpose(trf[0:1, 0:P], gmax[:], identity32[:])
            grow = big.tile([1, P], F32, tag="grow")
            nc.scalar.copy(grow[:], trf[0:1, 0:P])
            gsc = big.tile([1, 1], F32, tag="gsc")
            nc.vector.tensor_reduce(gsc[:], grow[:], axis=AX.X, op=OP.max)
            gre = big.tile([1, 1], F32, tag="gre")
            nc.scalar.activation(gre[:], gsc[:], ACT.Relu)
            gsq = big.tile([1, 1], F32, tag="gsq")
            nc.scalar.sqrt(gsq[:], gre[:])
            fin = big.tile([1, 8], F32, tag="fin")
            nc.vector.memset(fin[:], 0.0)
            nc.scalar.copy(fin[:, 0:1], gsq[:])
            nc.sync.dma_start(out=out_ext[:], in_=fin[:])

    nc.finalize()
    return nc


def shard_inputs(pred, target, pred_pad=PRED_PAD, tgt_pad=TGT_PAD, n_cores=N_CORES):
    pred = np.ascontiguousarray(pred, dtype=np.float32)
    target = np.ascontiguousarray(target, dtype=np.float32)
    n_pred = pred.shape[0]
    n_tgt = target.shape[0]
    per = (n_pred + n_cores - 1) // n_cores
    tpad = np.empty((tgt_pad, 4), np.float32)
    tpad[:n_tgt] = target
    tpad[n_tgt:] = target[0]  # duplicate targets never change a min
    in_maps = []
    for i in range(n_cores):
        lo = min(i * per, n_pred)
        hi = min(lo + per, n_pred)
        shard = np.empty((pred_pad, 4), np.float32)
        shard[: hi - lo] = pred[lo:hi]
        shard[hi - lo :] = pred[lo if hi > lo else 0]  # duplicate real rows
        in_maps.append({"pred": shard, "target": tpad})
    return in_maps


_NC_CACHE = {}


def kernel(pred, target, trace=False):
    global LAST_RESULT
    key = (PRED_PAD, TGT_PAD)
    if key not in _NC_CACHE:
        _NC_CACHE[key] = build_graph(*key)
    nc = _NC_CACHE[key]
    in_maps = shard_inputs(pred, target)
    res = run_bass_kernel_spmd(nc, in_maps, core_ids=list(range(N_CORES)), trace=trace)
    LAST_RESULT = res
    # host-side "all-reduce": max over the 8 per-core partial maxima
    val = max(float(res.results[i]["out"][0, 0]) for i in range(N_CORES))
    return np.array(val, dtype=np.float32)


# revision 42
# speedup vs baseline: 1.0289x; 1.0105x over previous
"""Asymmetric Hausdorff distance on 8 Trainium2 NeuronCores.

answer = max_i min_j ||pred[i,:3] - target[j,:3]||_2

Strategy (sharding_hint): shard pred rows across the 8 cores; each core
computes its row-block of the (implicit) distance matrix against the full
target set, takes per-row mins on the fly (never materializing the matrix),
then a global max over the 8 partial maxima (host-side, 8 scalars).

Per-core pipeline:
  d2(i,j) = |p_i|^2 + (|t_j|^2 - 2 p_i . t_j)
  s(i,j) = |t_j|^2 - 2 p_i . t_j is a K=11 bf16 matmul per (pred tile,
  target chunk), using a hi/lo bf16 split of each operand so the product
  is accurate to ~2^-16 (fp32 PSUM accumulate; the lo*lo term is dropped):
      lhsT rows: [a_hi(3), a_lo(3), a_hi(3), 1, 1]   a = -2*p
      rhs  rows: [t_hi(3), t_hi(3), t_lo(3), t2_hi, t2_lo]
  Matmuls run 4-way concurrent via tile_position row groups (targets are
  split into 4 subsets living at partition offsets 0/32/64/96), filling one
  4-bank PSUM tile per quad.  The rowwise min over PSUM is split across
  engines: some quads reduce directly on the Vector engine; the rest are
  drained by the Scalar engine to SBUF, pairwise-min'ed on GPSIMD, and
  min-reduced on Vector — balancing the three engines.  Then: add |p_i|^2,
  running max across pred tiles, cross-partition max via PE transpose,
  clamp+sqrt, one scalar out per core; host takes the max of 8.
"""

import numpy as np

import concourse.bass as bass
import concourse.mybir as mybir
import concourse.tile as tile
from concourse import bacc
from concourse.bass import ds
from concourse.bass_utils import run_bass_kernel_spmd
from concourse.masks import make_identity

F32 = mybir.dt.float32
BF16 = mybir.dt.bfloat16
AX = mybir.AxisListType
OP = mybir.AluOpType
ACT = mybir.ActivationFunctionType

N_CORES = 8
P = 128
KDIM = 11   # hi/lo split contraction: 3+3+3 products + t2_hi + t2_lo
MM_N = 512  # matmul moving chunk (one fp32 PSUM bank)
NSUB = 4    # concurrent row-group subsets (partition offsets 0/32/64/96)

# padded sizes: per-core pred rows (multiple of 128), target rows
# (multiple of NSUB*4*MM_N)
PRED_PAD = 3072   # 24 pred tiles of 128 per core
TGT_PAD = 24576   # 4 subsets x 12 chunks of 512; 12 quads per pred tile

# per-pred-tile quad schedule: which of the 12 quads reduce directly on DVE
# (fp32 from PSUM at 1 elem/cycle) vs go through the ACT-drain path (Scalar
# engine copies PSUM->SBUF bf16 with +|p|^2 bias, then DVE pairwise-mins at
# 2 elem/cycle in a tree and reduces once)
DIRECT_Q = (0, 6)
TREES_Q = ((1, 2, 3, 4, 5), (7, 8, 9, 10, 11))

LAST_RESULT = None  # BassKernelResults of the most recent run (for test.py)


def build_graph(pred_pad, tgt_pad, n_cores=N_CORES):
    assert pred_pad % P == 0
    assert tgt_pad % (NSUB * MM_N) == 0 and tgt_pad % P == 0
    n_ptiles = pred_pad // P
    n_tchunks = tgt_pad // P        # natural 128-row chunks
    n_quads = tgt_pad // (NSUB * MM_N)  # 4-bank PSUM tiles per pred tile
    if n_quads == 12:
        direct_q, trees_q = DIRECT_Q, TREES_Q
    elif n_quads >= 4:
        direct_q = (0,)
        trees_q = (tuple(range(1, n_quads)),)
    else:
        direct_q = tuple(range(n_quads))
        trees_q = ()

    nc = bacc.Bacc(trn_type="TRN2", num_devices=n_cores)

    pred_ext = nc.declare_dram_parameter("pred", [pred_pad, 4], F32, isOutput=False)
    tgt_ext = nc.declare_dram_parameter("target", [tgt_pad, 4], F32, isOutput=False)
    out_ext = nc.declare_dram_parameter("out", [1, 8], F32, isOutput=True)

    with tile.TileContext(nc) as tc:
        with (
            tc.tile_pool(name="big", bufs=1) as big,
            tc.tile_pool(name="work", bufs=3) as work,
            tc.tile_pool(name="drain", bufs=8) as drp,
            tc.tile_pool(name="pmain", bufs=2, space="PSUM") as pmain,
        ):
            identity = big.tile([P, P], BF16, tag="identity")
            make_identity(nc, identity[:])

            # ---- load inputs (row r of DRAM -> partition r // n_chunks,
            # chunk r % n_chunks: per-partition contiguous, fast DMA) ----
            pnat = big.tile([P, n_ptiles, 4], F32, tag="pnat")
            nc.sync.dma_start(
                out=pnat[:], in_=pred_ext[:].rearrange("(p c) k -> p c k", p=P)
            )
            tnat = big.tile([P, n_tchunks, 4], F32, tag="tnat")
            nc.sync.dma_start(
                out=tnat[:], in_=tgt_ext[:].rearrange("(p c) k -> p c k", p=P)
            )

            # ---- pred preprocessing ----
            # a = -2p; split a = a_hi + a_lo (bf16 each); p2 = |p|^2 (fp32)
            pa = big.tile([P, n_ptiles, 3], F32, tag="pa")
            nc.vector.tensor_scalar_mul(pa[:], pnat[:, :, 0:3], -2.0)
            pblk = big.tile([P, n_ptiles, NSUB, 32], BF16, tag="pblk")
            nc.gpsimd.memset(pblk[:].bitcast(F32), 0.0)
            pblk0 = pblk[:, :, 0, :]
            a_hi = pblk0[:, :, 0:3]
            nc.scalar.copy(a_hi, pa[:])                      # fp32 -> bf16 round
            pa_hi32 = big.tile([P, n_ptiles, 3], F32, tag="pa_hi32")
            nc.scalar.copy(pa_hi32[:], a_hi)                 # bf16 -> fp32 exact
            nc.vector.tensor_sub(pblk0[:, :, 3:6], pa[:], pa_hi32[:])  # a_lo
            nc.vector.tensor_copy(pblk0[:, :, 6:9], a_hi)     # a_hi again
            nc.vector.memset(pblk0[:, :, 9:11], 1.0)
            for g in range(1, NSUB):  # replicate K block to all row groups
                nc.gpsimd.tensor_copy(pblk[:, :, g, :], pblk0[:])
            psq = big.tile([P, n_ptiles, 3], F32, tag="psq")
            nc.vector.tensor_mul(psq[:], pnat[:, :, 0:3], pnat[:, :, 0:3])
            p2all = big.tile([P, n_ptiles], F32, tag="p2all")
            nc.vector.tensor_reduce(p2all[:], psq[:], axis=AX.X, op=OP.add)

            # ---- target preprocessing ----
            # t = t_hi + t_lo; t2 = |t|^2 = t2_hi + t2_lo (bf16 pairs)
            tblk = big.tile([P, n_tchunks, 32], BF16, tag="tblk")
            nc.gpsimd.memset(tblk[:].bitcast(F32), 0.0)
            t_hi = tblk[:, :, 0:3]
            nc.scalar.copy(t_hi, tnat[:, :, 0:3])
            t_hi32 = big.tile([P, n_tchunks, 3], F32, tag="t_hi32")
            nc.scalar.copy(t_hi32[:], t_hi)
            nc.vector.tensor_sub(tblk[:, :, 6:9], tnat[:, :, 0:3], t_hi32[:])  # t_lo
            nc.vector.tensor_copy(tblk[:, :, 3:6], t_hi)
            tsq = big.tile([P, n_tchunks, 3], F32, tag="tsq")
            nc.vector.tensor_mul(tsq[:], tnat[:, :, 0:3], tnat[:, :, 0:3])
            t2 = big.tile([P, n_tchunks], F32, tag="t2")
            nc.vector.tensor_reduce(t2[:], tsq[:], axis=AX.X, op=OP.add)
            t2_hi = tblk[:, :, 9:10]
            nc.scalar.copy(t2_hi, t2[:].rearrange("p (c o) -> p c o", o=1))
            t2_hi32 = big.tile([P, n_tchunks], F32, tag="t2_hi32")
            nc.scalar.copy(t2_hi32[:].rearrange("p (c o) -> p c o", o=1), t2_hi)
            nc.vector.tensor_sub(
                tblk[:, :, 10:11],
                t2[:].rearrange("p (c o) -> p c o", o=1),
                t2_hi32[:].rearrange("p (c o) -> p c o", o=1),
            )

            # ---- transpose to matmul layout via PE ----
            # lhsT_sb [128, pred_pad]: pred tile c at cols 128c..128c+127,
            #   K rows replicated at partition offsets 0/32/64/96.
            # rhs_sb [128, tgt_pad/4]: natural chunk c lives in subset
            #   g = c%4 (partition offset 32g), chunk-col c//4.
            # Each [128, 128] transpose covers FOUR chunks: the 11 K-values
            # sit at columns 0-10 of a 32-col block, so transposing a
            # [128, 4x32] input lands chunk g at partition offset 32g --
            # exactly the row-group layout the concurrent matmuls need.
            # Pred replicates one chunk across all 4 groups via a stride-0
            # broadcast AP.  Staging tiles hold 32 transposes (4-bank slot).
            lhsT_sb = big.tile([P, pred_pad], BF16, tag="lhsT")
            for b in range(0, n_ptiles, 32):
                nb = min(32, n_ptiles - b)
                tr = pmain.tile([P, 4096], BF16, tag="ps")
                for j in range(nb):
                    c = b + j
                    nc.tensor.transpose(
                        tr[:, j * P : (j + 1) * P],
                        pblk[:, c, :, :],
                        identity[:],
                    )
                nc.scalar.copy(
                    lhsT_sb[:, b * P : (b + nb) * P], tr[:, 0 : nb * P]
                )

            rhs_sb = big.tile([P, tgt_pad // NSUB], BF16, tag="rhs")
            n_cc = n_tchunks // NSUB
            for b in range(0, n_cc, 32):
                nb = min(32, n_cc - b)
                tr = pmain.tile([P, 4096], BF16, tag="ps")
                for j in range(nb):
                    cc = b + j
                    nc.tensor.transpose(
                        tr[:, j * P : (j + 1) * P],
                        tblk[:, NSUB * cc : NSUB * (cc + 1), :],
                        identity[:],
                    )
                nc.scalar.copy(
                    rhs_sb[:, b * P : (b + nb) * P], tr[:, 0 : nb * P]
                )

            # ---- main loop: 4-way concurrent matmuls + split min-reduce ----
            maxbuf = big.tile([P, n_ptiles], F32, tag="maxbuf")

            def quad_matmuls(c, q):
                ps = pmain.tile([P, NSUB * MM_N], F32, tag="ps")
                for g in range(NSUB):
                    nc.tensor.matmul(
                        ps[:, g * MM_N : (g + 1) * MM_N],
                        lhsT_sb[32 * g : 32 * g + KDIM, c * P : (c + 1) * P],
                        rhs_sb[32 * g : 32 * g + KDIM, q * MM_N : (q + 1) * MM_N],
                        start=True,
                        stop=True,
                        tile_position=(32 * g, 0),
                    )
                return ps

            bigA = big.tile([P, n_ptiles, max(len(direct_q), 1)], F32, tag="bigA")
            bigC = big.tile([P, n_ptiles, max(len(trees_q), 1)], F32, tag="bigC")
            for c in range(n_ptiles):
                p2c = p2all[:, ds(c, 1)]
                # route A: direct fp32 min-reduce from PSUM (values are s)
                # route C: ACT drains PSUM -> SBUF bf16 with +|p|^2 bias
                # (values are d2 >= 0, so bf16 keeps ~1e-3 relative), DVE
                # pairwise-mins at 2x in a tree, one bf16 reduce per tree.
                # Quads are emitted in index order (routes interleaved) so
                # the scheduler can overlap ACT drains with DVE reduces.
                minsA = bigA[:, c, :]
                minsC = bigC[:, c, :]
                tree_of_q = {}
                for ti, tree in enumerate(trees_q):
                    for q in tree:
                        tree_of_q[q] = ti
                levels = [[] for _ in trees_q]
                colA = 0
                for q in range(n_quads):
                    ps = quad_matmuls(c, q)
                    if q in direct_q:
                        nc.vector.tensor_reduce(
                            minsA[:, ds(colA, 1)], ps[:], axis=AX.X, op=OP.min
                        )
                        colA += 1
                        continue
                    ti = tree_of_q[q]
                    dr = drp.tile([P, NSUB * MM_N], BF16, tag="dr")
                    nc.scalar.activation(dr[:], ps[:], ACT.Identity, bias=p2c)
                    levels[ti].append(dr)
                    # combine pairs as soon as they are available
                    while len(levels[ti]) >= 2:
                        a, b = levels[ti][-2], levels[ti][-1]
                        mg = drp.tile([P, NSUB * MM_N], BF16, tag="mg")
                        nc.vector.tensor_tensor(mg[:], a[:], b[:], op=OP.min)
                        levels[ti] = levels[ti][:-2] + [mg]
                        if len(levels[ti]) < 2:
                            break
                for ti, tree in enumerate(trees_q):
                    level = levels[ti]
                    while len(level) > 1:
                        mg = drp.tile([P, NSUB * MM_N], BF16, tag="mg")
                        nc.vector.tensor_tensor(
                            mg[:], level[-2][:], level[-1][:], op=OP.min
                        )
                        level = level[:-2] + [mg]
                    nc.vector.tensor_reduce(
                        minsC[:, ds(ti, 1)], level[0][:], axis=AX.X, op=OP.min
                    )
                pass

            # batched combine: d2[p, c] = min(min_A + p2, min_C)
            dA = big.tile([P, n_ptiles], F32, tag="dA")
            nc.vector.tensor_reduce(dA[:], bigA[:], axis=AX.X, op=OP.min)
            nc.vector.tensor_add(dA[:], dA[:], p2all[:])
            if trees_q:
                dC = big.tile([P, n_ptiles], F32, tag="dC")
                nc.vector.tensor_reduce(dC[:], bigC[:], axis=AX.X, op=OP.min)
                nc.vector.tensor_tensor(maxbuf[:], dA[:], dC[:], op=OP.min)
            else:
                nc.vector.tensor_copy(maxbuf[:], dA[:])

            # ---- finalize: max over partitions (fp32 PE transpose), clamp,
            # sqrt; one scalar per core, host maxes the 8 ----
            identity32 = big.tile([P, P], F32, tag="identity32")
            make_identity(nc, identity32[:])
            gmax = big.tile([P, 1], F32, tag="gmax")
            nc.vector.tensor_reduce(gmax[:], maxbuf[:], axis=AX.X, op=OP.max)
            trf = pmain.tile([P, 512], F32, tag="ps")
            nc.tensor.transpose(trf[0:1, 0:P], gmax[:], identity32[:])
            grow = big.tile([1, P], F32, tag="grow")
            nc.scalar.copy(grow[:], trf[0:1, 0:P])
            gsc = big.tile([1, 1], F32, tag="gsc")
            nc.vector.tensor_reduce(gsc[:], grow[:], axis=AX.X, op=OP.max)
            gre = big.tile([1, 1], F32, tag="gre")
            nc.scalar.activation(gre[:], gsc[:], ACT.Relu)
            gsq = big.tile([1, 1], F32, tag="gsq")
            nc.scalar.sqrt(gsq[:], gre[:])
            fin = big.tile([1, 8], F32, tag="fin")
            nc.vector.memset(fin[:], 0.0)
            nc.scalar.copy(fin[:, 0:1], gsq[:])
            nc.sync.dma_start(out=out_ext[:], in_=fin[:])

    nc.finalize()
    return nc


def shard_inputs(pred, target, pred_pad=PRED_PAD, tgt_pad=TGT_PAD, n_cores=N_CORES):
    pred = np.ascontiguousarray(pred, dtype=np.float32)
    target = np.ascontiguousarray(target, dtype=np.float32)
    n_pred = pred.shape[0]
    n_tgt = target.shape[0]
    per = (n_pred + n_cores - 1) // n_cores
    tpad = np.empty((tgt_pad, 4), np.float32)
    tpad[:n_tgt] = target
    tpad[n_tgt:] = target[0]  # duplicate targets never change a min
    in_maps = []
    for i in range(n_cores):
        lo = min(i * per, n_pred)
        hi = min(lo + per, n_pred)
        shard = np.empty((pred_pad, 4), np.float32)
        shard[: hi - lo] = pred[lo:hi]
        shard[hi - lo :] = pred[lo if hi > lo else 0]  # duplicate real rows
        in_maps.append({"pred": shard, "target": tpad})
    return in_maps


_NC_CACHE = {}


def kernel(pred, target, trace=False):
    global LAST_RESULT
    key = (PRED_PAD, TGT_PAD)
    if key not in _NC_CACHE:
        _NC_CACHE[key] = build_graph(*key)
    nc = _NC_CACHE[key]
    in_maps = shard_inputs(pred, target)
    res = run_bass_kernel_spmd(nc, in_maps, core_ids=list(range(N_CORES)), trace=trace)
    LAST_RESULT = res
    # host-side "all-reduce": max over the 8 per-core partial maxima
    val = max(float(res.results[i]["out"][0, 0]) for i in range(N_CORES))
    return np.array(val, dtype=np.float32)
